# revision 18
# baseline (speedup 1.0000x reference)
"""2-layer GCN (PyG GCNConv x2 + ReLU) on 8 Trainium2 NeuronCores.

out = Ahat @ relu(Ahat @ X @ W1 + b1) @ W2 + b2,  Ahat = D^-1/2 (A+I) D^-1/2

Strategy (destination-sharded, graph-parallel):
  - Host: shard destination nodes across 8 cores (2500 each); per core, sort
    incoming edges (+self loops) by destination, pack into 128-edge chunks per
    128-destination tile, and build one-hot fp16 selection matrices S so that
    segment-sum aggregation becomes PSUM-accumulated matmuls (race-free, exact
    fp32 accumulation).  The symmetric normalization is folded into the node
    features (rows pre-scaled by D^-1/2 on the host) and fused destination-side
    scales on the DVE.
  - Associativity: (A+I)(Xs) @ W1 aggregates the *input* features first, so
    layer-1 gathers run against the replicated input table from t=0 with no
    preliminary matmul or collective; the W1/W2 matmuls run post-aggregation
    on each core's 2500-node shard only.
  - Device, per core: L1 aggregation (dma_gather rows of Xs + S matmuls,
    descriptor generation spread over SWDGE queues 1-3 = 3 Q7 pairs) ->
    per-tile epilogue (scale, PE transpose, @W1, relu-scale, transpose, @W2)
    -> AllGather y2 (compact fp16) -> L2 aggregation (same S/indices) ->
    final D^-1/2 scale (+bias) -> output shard, fp32.
"""

import sys

sys.path.insert(0, "/opt/trn_rl_repo")

import numpy as np

import concourse.bacc as bacc
import concourse.tile as tile
import concourse.mybir as mybir
from concourse import bass_utils

N_CORES = 8
N_NODES = 20000
IN_CH = 256
HID_CH = 256
OUT_CH = 128
SHARD = N_NODES // N_CORES  # 2500
P = 128
N_TILES = (SHARD + P - 1) // P  # 20
BATCH_CHUNKS = 16  # edge chunks per dma_gather call (2048 rows)
GATHER_QUEUES = (0, 1, 2, 3)  # round-robin descgen across all 4 Q7 pairs

F16 = mybir.dt.float16
F32 = mybir.dt.float32
I16 = mybir.dt.int16


def _host_prep(doc_embeds, edge_index, W1, b1, W2, b2):
    X = np.asarray(doc_embeds, np.float32)
    ei = np.asarray(edge_index)
    src_g = ei[0].astype(np.int64)
    dst_g = ei[1].astype(np.int64)

    deg = np.bincount(dst_g, minlength=N_NODES).astype(np.float32) + 1.0
    dis = 1.0 / np.sqrt(deg)  # [N]

    xsfull = np.ascontiguousarray((X * dis[:, None]).astype(np.float16))  # [N, 256]
    W1h = np.ascontiguousarray(np.asarray(W1, np.float16))  # [256, 256]
    W2h = np.ascontiguousarray(np.asarray(W2, np.float16))  # [256, 128]

    core_of = dst_g // SHARD
    per_core = []
    counts = np.zeros((N_CORES, N_TILES), np.int64)
    for m in range(N_CORES):
        sel = np.nonzero(core_of == m)[0]
        s = src_g[sel]
        d = dst_g[sel] - m * SHARD
        # merge parallel edges: S carries the multiplicity; self loops are
        # handled separately as an identity matmul on the contiguous shard
        key = d * np.int64(N_NODES) + s
        uk, w = np.unique(key, return_counts=True)
        s = uk % N_NODES
        d = uk // N_NODES
        per_core.append((s, d, w.astype(np.float32)))
        counts[m] = np.bincount(d // P, minlength=N_TILES)

    # uniform per-tile chunk counts across cores (SPMD: same program everywhere)
    C_t = np.maximum((counts.max(axis=0) + P - 1) // P, 1).astype(np.int64)
    # process tiles with many chunks first so the tail tile is cheap
    tile_order = np.argsort(-C_t, kind="stable").astype(np.int64)
    pos_of_tile = np.empty(N_TILES, np.int64)
    pos_of_tile[tile_order] = np.arange(N_TILES)
    C_sched = C_t[tile_order]
    sched_offsets = np.concatenate([[0], np.cumsum(C_sched)])
    offsets = sched_offsets[pos_of_tile]  # chunk offset per physical tile
    sumC = int(C_t.sum())
    L = sumC * P

    srcs = np.zeros((N_CORES, L), np.int64)
    shost = np.zeros((N_CORES, P, L), np.float16)
    for m in range(N_CORES):
        s, d, w = per_core[m]
        tile_of = d // P
        first = np.searchsorted(d, np.arange(N_TILES) * P, side="left")
        rank = np.arange(len(d)) - first[tile_of]
        pos = offsets[tile_of] * P + rank
        srcs[m, pos] = s
        g = pos // P
        p = pos % P
        dloc = d - tile_of * P
        shost[m, p, g * P + dloc] = w.astype(np.float16)

    # wrapped int16 index layout: index i -> [16*grp + i%16, i//16], 8 replicas
    idxw = np.empty((N_CORES, P, L // 16), np.int16)
    for m in range(N_CORES):
        base = srcs[m].astype(np.int16).reshape(L // 16, 16).T  # [16, L//16]
        idxw[m] = np.tile(base, (8, 1))

    # per-partition scale tiles [128, N_TILES]: node m*SHARD + t*128 + p
    dist = np.zeros((N_CORES, P, N_TILES), np.float32)
    pad = N_TILES * P - SHARD
    for m in range(N_CORES):
        dsh = np.pad(dis[m * SHARD : (m + 1) * SHARD], (0, pad))
        dist[m] = dsh.reshape(N_TILES, P).T

    b1f = np.asarray(b1, np.float32)
    b2f = np.asarray(b2, np.float32)
    has_b1 = bool(np.any(b1f))
    has_b2 = bool(np.any(b2f))
    b1bc = np.broadcast_to(b1f, (P, HID_CH)).copy()
    b2bc = np.broadcast_to(b2f, (P, OUT_CH)).copy()

    identity = np.eye(P, dtype=np.float16)
    in_maps = []
    for m in range(N_CORES):
        im = {
            "xsfull": xsfull,
            "xsown": np.ascontiguousarray(xsfull[m * SHARD : (m + 1) * SHARD]),
            "ident": identity,
            "w1": W1h,
            "w2": W2h,
            "sfull": np.ascontiguousarray(shost[m]),
            "idxall": np.ascontiguousarray(idxw[m]),
            "dist": np.ascontiguousarray(dist[m]),
        }
        if has_b1:
            im["b1bc"] = b1bc
        if has_b2:
            im["b2bc"] = b2bc
        in_maps.append(im)
    meta = dict(C_t=C_t, offsets=offsets, sumC=sumC, L=L, has_b1=has_b1,
                has_b2=has_b2, tile_order=tile_order)
    return in_maps, meta


def _build_program(meta):
    offsets = meta["offsets"]
    sumC = meta["sumC"]
    L = meta["L"]
    has_b1 = meta["has_b1"]
    has_b2 = meta["has_b2"]
    # batch schedule over the chunk list: small batches at the ramp (fill the
    # four Q7 pairs quickly) and at the tail (short drain before the collective)
    batches = []
    c = 0
    while c < sumC:
        size = min(8, sumC - c)
        batches.append((c, c + size))
        c += size
    n_batches = len(batches)
    batch_of_chunk = np.zeros(sumC, np.int64)
    for bi, (c0, c1) in enumerate(batches):
        batch_of_chunk[c0:c1] = bi

    nc = bacc.Bacc(
        "TRN2",
        target_bir_lowering=False,
        debug=False,
        num_devices=N_CORES,
        num_swdge_queues=4,
        dynamic_dma_scratch_size=32768,
    )

    xs_d = nc.dram_tensor("xsfull", [N_NODES, IN_CH], F16, kind="ExternalInput").ap()
    xso_d = nc.dram_tensor("xsown", [SHARD, IN_CH], F16, kind="ExternalInput").ap()
    id_d = nc.dram_tensor("ident", [P, P], F16, kind="ExternalInput").ap()
    w1 = nc.dram_tensor("w1", [IN_CH, HID_CH], F16, kind="ExternalInput").ap()
    w2 = nc.dram_tensor("w2", [HID_CH, OUT_CH], F16, kind="ExternalInput").ap()
    sfull_d = nc.dram_tensor("sfull", [P, L], F16, kind="ExternalInput").ap()
    idx_d = nc.dram_tensor("idxall", [P, L // 16], I16, kind="ExternalInput").ap()
    dist_d = nc.dram_tensor("dist", [P, N_TILES], F32, kind="ExternalInput").ap()
    b1_d = b2_d = None
    if has_b1:
        b1_d = nc.dram_tensor("b1bc", [P, HID_CH], F32, kind="ExternalInput").ap()
    if has_b2:
        b2_d = nc.dram_tensor("b2bc", [P, OUT_CH], F32, kind="ExternalInput").ap()
    out_d = nc.dram_tensor("out", [SHARD, OUT_CH], F32, kind="ExternalOutput").ap()

    rg = [list(range(N_CORES))]

    with tile.TileContext(nc) as tc:
        with (
            tc.tile_pool(name="dram", bufs=1, space="DRAM") as dram,
            tc.tile_pool(name="const", bufs=1) as cpool,
            tc.tile_pool(name="sseg", bufs=1) as spool,
            tc.tile_pool(name="gat", bufs=10) as gpool,
            tc.tile_pool(name="work", bufs=2) as wpool,
            tc.tile_pool(name="psa", bufs=4, space="PSUM") as ps_agg,
            tc.tile_pool(name="pst", bufs=2, space="PSUM") as ps_tr,
            tc.tile_pool(name="pso", bufs=2, space="PSUM") as ps_o,
        ):
            # ---- indices + S segments first (gathers depend only on these) ----
            idxt = cpool.tile([P, L // 16], I16)
            nc.sync.dma_start(out=idxt[:], in_=idx_d[:])

            stiles = []
            for b, (c0, c1) in enumerate(batches):
                st = spool.tile([P, (c1 - c0) * P], F16, name=f"sseg{b}", tag=f"sseg{b}")
                nc.sync.dma_start(out=st[:], in_=sfull_d[:, c0 * P : c1 * P])
                stiles.append(st)

            w1t = cpool.tile([P, 2, HID_CH], F16)
            w2t = cpool.tile([P, 2, OUT_CH], F16)
            for k in range(2):
                nc.scalar.dma_start(out=w1t[:, k, :], in_=w1[k * P : (k + 1) * P, :])
                nc.scalar.dma_start(out=w2t[:, k, :], in_=w2[k * P : (k + 1) * P, :])
            distt = cpool.tile([P, N_TILES], F32)
            nc.scalar.dma_start(out=distt[:], in_=dist_d[:])
            ident = cpool.tile([P, P], F16)
            nc.scalar.dma_start(out=ident[:], in_=id_d[:])
            b1t = b2t = None
            if has_b1:
                b1t = cpool.tile([P, HID_CH], F32)
                nc.sync.dma_start(out=b1t[:], in_=b1_d[:])
            if has_b2:
                b2t = cpool.tile([P, OUT_CH], F32)
                nc.sync.dma_start(out=b2t[:], in_=b2_d[:])

            # ---- DRAM intermediates ----
            y2own = dram.tile([SHARD, OUT_CH], F16)
            y2full = dram.tile([N_NODES, OUT_CH], F16, addr_space="Shared")

            # ---- aggregation helper (both layers) ----
            swdge_ctr = [0]  # SWDGE DMA emission counter (lane/queue pairing)

            selfp = None

            def aggregate(src_full, width, epilogue, phase, self_src):
                """Segment-sum src_full rows by destination tile via S matmuls.

                epilogue(t, tw, psum_tile) consumes the [128, width] fp32 sums.
                """
                gts = {}

                def ensure_batch(b):
                    if b in gts:
                        return
                    c0, c1 = batches[b]
                    nch = c1 - c0
                    gt = gpool.tile(
                        [P, 8, width], F16, name=f"g{phase}_{b}", tag="gat"
                    )
                    # Tile hands SWDGE DMAs their DMASW sem lane round-robin
                    # (mod 8) in program order; keep queue = emission mod 4 so
                    # every lane is only ever fed from one queue (lane FIFO
                    # order == completion order, no cross-queue sem races).
                    q = GATHER_QUEUES[swdge_ctr[0] % len(GATHER_QUEUES)]
                    swdge_ctr[0] += 1
                    nc.gpsimd.dma_gather(
                        out_ap=gt[:, :nch, :],
                        in_ap=src_full[:],
                        idxs_ap=idxt[:, c0 * 8 : c1 * 8],
                        num_idxs=nch * P,
                        num_idxs_reg=nch * P,
                        elem_size=width,
                        single_packet=False,
                        queue_num=q,
                    )
                    gts[b] = gt

                for t in meta["tile_order"]:
                    t = int(t)
                    n0 = t * P
                    tw = min(P, SHARD - n0)
                    ps = ps_agg.tile([P, HID_CH], F32, name="psagg", tag="psagg")
                    # self-loop term: contiguous rows of our own shard via an
                    # identity matmul (one cheap DMA instead of 128 gathers).
                    # Issued on the scalar HWDGE ring so it is not queued
                    # behind the bulk S-segment loads on the sync ring.
                    sst = wpool.tile([P, width], F16, name=f"sst{phase}", tag="sst")
                    nc.scalar.dma_start(out=sst[:tw, :], in_=self_src(n0, tw))
                    g0 = int(offsets[t])
                    g1 = g0 + int(meta["C_t"][t])
                    for g in range(g0, g1):
                        b = int(batch_of_chunk[g])
                        ensure_batch(b)
                        gl = g - batches[b][0]
                        nc.tensor.matmul(
                            ps[:, :width],
                            lhsT=stiles[b][:, gl * P : (gl + 1) * P],
                            rhs=gts[b][:, gl, :],
                            start=(g == g0),
                            stop=False,
                        )
                    # K=tw keeps the stale rows of sst out of the contraction
                    nc.tensor.matmul(
                        ps[:, :width],
                        lhsT=ident[:tw, :],
                        rhs=sst[:tw, :],
                        start=False,
                        stop=True,
                    )
                    epilogue(t, tw, ps)

            # transpose [128, 2*P_cols] fp16 SBUF tile -> [128, n_k, P] fp16
            def transpose2(x_sb, n_k, name):
                xT = wpool.tile([P, n_k, P], F16, name=name, tag=name)
                for k in range(n_k):
                    pst = ps_tr.tile([P, P], F16, name="pst", tag="pst")
                    nc.tensor.transpose(
                        out=pst[:], in_=x_sb[:, k * P : (k + 1) * P], identity=ident[:]
                    )
                    nc.vector.tensor_copy(out=xT[:, k, :], in_=pst[:])
                return xT

            # ---- phase B: L1 aggregation -> out1 -> x1s -> y2 ----
            def epilogue1(t, tw, ps):
                # u = dis * aggX   [128, 256] fp16
                u_sb = wpool.tile([P, IN_CH], F16, name="u_sb", tag="u_sb")
                nc.vector.tensor_scalar(
                    out=u_sb[:],
                    in0=ps[:, :IN_CH],
                    scalar1=distt[:, t : t + 1],
                    scalar2=None,
                    op0=mybir.AluOpType.mult,
                )
                uT = transpose2(u_sb, 2, "uT")
                pso1 = ps_o.tile([P, HID_CH], F32, name="pso1", tag="pso")
                for k in range(2):
                    nc.tensor.matmul(
                        pso1[:],
                        lhsT=uT[:, k, :],
                        rhs=w1t[:, k, :],
                        start=(k == 0),
                        stop=(k == 1),
                    )
                # x1s = dis * relu(out1 + b1)
                x1s = wpool.tile([P, HID_CH], F16, name="x1s", tag="x1s")
                if not has_b1:
                    nc.vector.tensor_scalar(
                        out=x1s[:],
                        in0=pso1[:],
                        scalar1=0.0,
                        scalar2=distt[:, t : t + 1],
                        op0=mybir.AluOpType.max,
                        op1=mybir.AluOpType.mult,
                    )
                else:
                    tmp = wpool.tile([P, HID_CH], F32, name="tmpb1", tag="tmpb1")
                    nc.vector.tensor_tensor(
                        out=tmp[:], in0=pso1[:], in1=b1t[:], op=mybir.AluOpType.add
                    )
                    nc.vector.tensor_scalar(
                        out=x1s[:],
                        in0=tmp[:],
                        scalar1=0.0,
                        scalar2=distt[:, t : t + 1],
                        op0=mybir.AluOpType.max,
                        op1=mybir.AluOpType.mult,
                    )
                x1sT = transpose2(x1s, 2, "x1sT")
                psy2 = ps_o.tile([P, OUT_CH], F32, name="psy2", tag="pso")
                for k in range(2):
                    nc.tensor.matmul(
                        psy2[:],
                        lhsT=x1sT[:, k, :],
                        rhs=w2t[:, k, :],
                        start=(k == 0),
                        stop=(k == 1),
                    )
                y2sb = wpool.tile([P, OUT_CH], F16, name="y2sb", tag="y2sb")
                nc.vector.tensor_copy(out=y2sb[:tw, :], in_=psy2[:tw, :])
                n0 = t * P
                nc.sync.dma_start(out=y2own[n0 : n0 + tw, :], in_=y2sb[:tw, :])

            aggregate(xs_d, IN_CH, epilogue1, phase=0,
                      self_src=lambda n0, tw: xso_d[n0 : n0 + tw, :])

            nc.gpsimd.collective_compute(
                "AllGather",
                mybir.AluOpType.bypass,
                replica_groups=rg,
                ins=[y2own.opt()],
                outs=[y2full.opt()],
            )

            # ---- phase C: L2 aggregation -> out ----
            def epilogue2(t, tw, ps):
                outsb = wpool.tile([P, OUT_CH], F32, name="outsb", tag="outsb")
                nc.vector.tensor_scalar(
                    out=outsb[:],
                    in0=ps[:, :OUT_CH],
                    scalar1=distt[:, t : t + 1],
                    scalar2=None,
                    op0=mybir.AluOpType.mult,
                )
                if has_b2:
                    nc.vector.tensor_tensor(
                        out=outsb[:], in0=outsb[:], in1=b2t[:], op=mybir.AluOpType.add
                    )
                n0 = t * P
                nc.sync.dma_start(out=out_d[n0 : n0 + tw, :], in_=outsb[:tw, :])

            aggregate(y2full, OUT_CH, epilogue2, phase=1,
                      self_src=lambda n0, tw: y2own[n0 : n0 + tw, :])

    nc.compile()
    return nc


def run(inputs, trace=False, trace_kwargs=None):
    """Build, run on 8 cores, return (output, BassKernelResults)."""
    in_maps, meta = _host_prep(**inputs)
    nc = _build_program(meta)
    res = bass_utils.run_bass_kernel_spmd(
        nc,
        in_maps,
        core_ids=list(range(N_CORES)),
        trace=trace,
        **(trace_kwargs or {}),
    )
    out = np.concatenate([res.results[m]["out"] for m in range(N_CORES)], axis=0)
    return out, res


def kernel(**inputs) -> np.ndarray:
    out, _ = run(inputs)
    return out
